# revision 14
# baseline (speedup 1.0000x reference)
"""MoE CouncilLayer kernel for 8x TRN2 NeuronCores (expert-parallel).

Problem (all-expert MoE, B=2, T=1024, C=768, E=32, H=3072):
    gates = softmax(x @ gate_w + gate_b)                     # [N, E]
    h     = gelu(einsum('nc,ech->neh', x, w1) + b1)          # [N, E, H]
    y     = einsum('neh,ehc->nec', h, w2) + b2               # [N, E, C]
    out   = einsum('ne,nec->nc', gates, y)                   # [N, C]

Sharding: expert-parallel, 4 experts per core; x replicated. Each core
computes its 4 experts' gate-weighted partial sum; host adds the 8
partials. Gates are computed host-side (they are needed on host anyway
to weight b2, and they are off the device critical path entirely).

Arithmetic: 3-term hi/lo fp8 with DoubleRow perf mode. Every matmul
operand A is split host-side (or on-device for h) into e4m3 hi+lo
parts, A = Ah + Al + O(0.07% of A). The product A@B is computed as
    Ah@Bh  +  (Al@Bh + Ah@Bl)        [lo@lo dropped, ~1e-4 relative]
where the two cross terms fit in ONE DoubleRow matmul instruction
(DoubleRow computes lhsT[:,0].T @ rhs[:,0] + lhsT[:,1].T @ rhs[:,1]),
and hi@hi terms pair ADJACENT contraction chunks in one instruction.
Net: 1.5 DoubleRow instructions per 128-contraction chunk, at 0.5
cycles/output-column each = 0.75x the fp16 PE cost. End-to-end rel
err ~2e-3 (vs 2e-2 budget): mm1/mm2 operand residuals ~0.07% each.

Scaling: w1 is scaled by 32 and w2 by 64 host-side so the lo residues
land in e4m3 normal range; mm1 descales via the ACT gelu scale
(gelu(psum/32 + b1)) and mm2 descales via host-prescaled gates
(g/64), applied after mm2 on DVE:  yac += (g/64) * yps.

Per-core layout is feature-major (activations stored [feature, token]).
hi/lo parts are interleaved in one tile dim so both DoubleRow operand
pairings are single strided APs:
    x8  [128, cc, 2, N]   dim2: 0=hi 1=lo
    w1t [128, cc, 2, 512] dim2: 0=lo 1=hi   (cross pair = [:,cc,0:2,:])
    hg8 [128, hb, 2, TG]  dim2: 0=hi 1=lo
    w2t [128, hb, 2, 128] dim2: 0=lo 1=hi
h split: ACT emits gelu twice from the mm1 psum (fp16 tmp + fp8 hi),
DVE tensor_sub produces the fp8 lo. Engine budget per core: PE ~738us
(bottleneck), ACT ~490us, DVE ~370us, DMA ~44MB ~130us.
"""

import numpy as np
import ml_dtypes

import concourse.tile as tile
from concourse import bacc, mybir
from concourse.bass_utils import run_bass_kernel_spmd

# Problem dims (hardcoded per harness contract)
B, T, C, E, H = 2, 1024, 768, 32, 3072
N = B * T  # 2048 tokens
NCORES = 8
EL = E // NCORES  # 4 local experts
CB = C // 128  # 6 c-blocks (3 DoubleRow pairs)
HB = H // 128  # 24 h-blocks (12 DoubleRow pairs)
TCG = 2  # token groups (1024 each)
TG = N // TCG  # 1024
TI = TG // 512  # 512-token chunks per group

S1 = 32.0  # host-side w1 scale
S2 = 64.0  # host-side w2 scale

F8 = mybir.dt.float8e4
F16 = mybir.dt.float16
F32 = mybir.dt.float32
AF = mybir.ActivationFunctionType
DR = mybir.MatmulPerfMode.DoubleRow
E4NP = ml_dtypes.float8_e4m3

_CACHED_NC = None


def build_nc(act=AF.Gelu):
    nc = bacc.Bacc(trn_type="TRN2")

    # tile-layout packed inputs (host pre-arranges so DMAs are contiguous):
    # x8: [128, tg, cc, 2(hi,lo), TG];  w1: [EL, hbg, 128, cc*2(lo,hi)*512]
    # w2: [EL, cb, 128, hb*2(lo,hi)*128]
    x8_d = nc.dram_tensor("x8", [128, TCG, CB, 2, TG], F8, kind="ExternalInput")
    g_d = nc.dram_tensor("g", [EL, N], F16, kind="ExternalInput")
    w1_d = nc.dram_tensor("w1", [EL, HB // 4, 128, CB * 2 * 512], F8, kind="ExternalInput")
    b1_d = nc.dram_tensor("b1", [128, EL, HB], F32, kind="ExternalInput")
    w2_d = nc.dram_tensor("w2", [EL, CB, 128, HB * 2 * 128], F8, kind="ExternalInput")
    outT_d = nc.dram_tensor("outT", [C, N], F32, kind="ExternalOutput")

    with tile.TileContext(nc) as tc:
        with (
            tc.tile_pool(name="const", bufs=1) as cp,
            tc.tile_pool(name="stream", bufs=1) as sp,
            tc.tile_pool(name="psum", bufs=1, space="PSUM") as pp,
        ):
            # --- resident tiles ---
            x8_sb = cp.tile([128, TCG, CB, 2, TG], F8)
            g_sb = cp.tile([128, EL, N], F16)
            b1_sb = cp.tile([128, EL, HB], F32)

            # DMA issue order = arrival order. The specially-traced first
            # mm1 block consumes x cc-chunks as they land; its w1 tile and
            # the first x chunks go first, b1 (needed by the first gelu)
            # rides after the first group, then the rest of x tg0, the
            # second w1 tile prefetch, x tg1, and the gate broadcasts.
            w1t_first = sp.tile([128, CB, 2, 512], F8, tag="w1", bufs=4, name="w1t")
            w1f_ap = w1_d[0, 0, :, :].rearrange(
                "p (cc two h) -> p cc two h", cc=CB, two=2
            )
            # finest-first interleave: the opening matmul needs only x cc0
            # (first 512 tokens) + w1 cc0; each later group keeps PE just
            # behind the arrival stream through the cc-outer first block.
            nc.sync.dma_start(x8_sb[:, 0, 0, :, 0:512], x8_d[:, 0, 0, :, 0:512])
            nc.sync.dma_start(w1t_first[:, 0, :, :], w1f_ap[:, 0, :, :])
            nc.sync.dma_start(x8_sb[:, 0, 0, :, 512:TG], x8_d[:, 0, 0, :, 512:TG])
            nc.sync.dma_start(x8_sb[:, 0, 1, :, :], x8_d[:, 0, 1, :, :])
            nc.sync.dma_start(w1t_first[:, 1, :, :], w1f_ap[:, 1, :, :])
            nc.sync.dma_start(x8_sb[:, 0, 2:4, :, :], x8_d[:, 0, 2:4, :, :])
            nc.sync.dma_start(w1t_first[:, 2:4, :, :], w1f_ap[:, 2:4, :, :])
            nc.sync.dma_start(x8_sb[:, 0, 4:6, :, :], x8_d[:, 0, 4:6, :, :])
            nc.sync.dma_start(w1t_first[:, 4:6, :, :], w1f_ap[:, 4:6, :, :])
            nc.sync.dma_start(b1_sb, b1_d[:, :, :])
            # prefetch e0's second w1 tile; x tg1 and the gate broadcasts are
            # deferred into the main loop (they are not needed until ~50us in
            # and would delay e0's hbg2+ weight tiles here).
            w1t_second = sp.tile([128, CB, 2, 512], F8, tag="w1", bufs=4, name="w1t")
            nc.sync.dma_start(w1t_second, w1_d[0, 1, :, :])

            def emit_mm1_tile(ps, w1t, hbi, tg, lts, start_first):
                # 9 DoubleRow instrs: 6 cross (per cc) + 3 hi-hi (per pair)
                hs = slice(hbi * 128, (hbi + 1) * 128)
                for cc in range(CB):
                    nc.tensor.matmul(
                        ps,
                        w1t[:, cc, 0:2, hs],
                        x8_sb[:, tg, cc, 0:2, lts],
                        start=(cc == 0) and start_first,
                        stop=False,
                        perf_mode=DR,
                    )
                for p in range(CB // 2):
                    nc.tensor.matmul(
                        ps,
                        w1t[:, 2 * p : 2 * p + 2, 1, hs],
                        x8_sb[:, tg, 2 * p : 2 * p + 2, 0, lts],
                        start=False,
                        stop=(p == CB // 2 - 1),
                        perf_mode=DR,
                    )

            def emit_gelu_split(tg, e, hb, hps, hg8):
                # ACT: gelu from psum -> fp16; Pool(gpsimd): cast fp16 -> fp8
                # hi part; DVE: subtract -> fp8 lo part. One pass per engine
                # per tile keeps all three under PE's 960ns/tile mm1 pace.
                for ti in range(TI):
                    lts = slice(ti * 512, (ti + 1) * 512)
                    h16 = sp.tile([128, 512], F16, tag="h16", bufs=4, name="h16")
                    nc.scalar.activation(
                        h16, hps[ti], act, bias=b1_sb[:, e, hb : hb + 1], scale=1.0 / S1
                    )
                    nc.gpsimd.tensor_copy(hg8[:, hb, 0, lts], h16)
                    nc.vector.tensor_sub(hg8[:, hb, 1, lts], h16, hg8[:, hb, 0, lts])

            def emit_mm1_first(hg8):
                # first 4 h-blocks of (tg0, e0), traced cc-OUTER across all 8
                # psum banks so PE consumes each arriving x chunk immediately;
                # hi-hi terms stitched in after each odd cc completes a pair.
                hps8 = [
                    [
                        pp.tile(
                            [128, 512],
                            F32,
                            tag=("h" if hbi < 2 else "y"),
                            bufs=4,
                            name="hps",
                        )
                        for _ in range(TI)
                    ]
                    for hbi in range(4)
                ]
                for cc in range(CB):
                    for ti in range(TI):
                        lts = slice(ti * 512, (ti + 1) * 512)
                        for hbi in range(4):
                            nc.tensor.matmul(
                                hps8[hbi][ti],
                                w1t_first[:, cc, 0:2, hbi * 128 : (hbi + 1) * 128],
                                x8_sb[:, 0, cc, 0:2, lts],
                                start=(cc == 0),
                                stop=False,
                                perf_mode=DR,
                            )
                    if cc % 2 == 1:
                        p = cc // 2
                        for ti in range(TI):
                            lts = slice(ti * 512, (ti + 1) * 512)
                            for hbi in range(4):
                                nc.tensor.matmul(
                                    hps8[hbi][ti],
                                    w1t_first[:, cc - 1 : cc + 1, 1, hbi * 128 : (hbi + 1) * 128],
                                    x8_sb[:, 0, cc - 1 : cc + 1, 0, lts],
                                    start=False,
                                    stop=(p == CB // 2 - 1),
                                    perf_mode=DR,
                                )
                for hbi in range(4):
                    emit_gelu_split(0, 0, hbi, hps8[hbi], hg8)

            def emit_mm1(tg, e, hg8, hbg_start=0):
                for hbg in range(hbg_start, HB // 4):
                    if tg == 0 and e == 0 and hbg == 1:
                        w1t = w1t_second
                    else:
                        w1t = sp.tile(
                            [128, CB, 2, 512], F8, tag="w1", bufs=4, name="w1t"
                        )
                        nc.sync.dma_start(w1t, w1_d[e, hbg, :, :])
                    for hbi in range(4):
                        hb = hbg * 4 + hbi
                        hps = [
                            pp.tile([128, 512], F32, tag="h", bufs=4, name="hps")
                            for _ in range(TI)
                        ]
                        for ti in range(TI):
                            lts = slice(ti * 512, (ti + 1) * 512)
                            emit_mm1_tile(hps[ti], w1t, hbi, tg, lts, True)
                        emit_gelu_split(tg, e, hb, hps, hg8)

            def emit_mm2(tg, e, hg8, yac):
                for cb in range(CB):
                    w2t = sp.tile([128, HB, 2, 128], F8, tag="w2", bufs=3, name="w2t")
                    nc.sync.dma_start(w2t, w2_d[e, cb, :, :])
                    yps = [
                        pp.tile([128, 512], F32, tag="y", bufs=4, name="yps")
                        for _ in range(TI)
                    ]
                    for ti in range(TI):
                        lts = slice(ti * 512, (ti + 1) * 512)
                        for hb in range(HB):
                            nc.tensor.matmul(
                                yps[ti],
                                w2t[:, hb, 0:2, :],
                                hg8[:, hb, 0:2, lts],
                                start=(hb == 0),
                                stop=False,
                                perf_mode=DR,
                            )
                        for p in range(HB // 2):
                            nc.tensor.matmul(
                                yps[ti],
                                w2t[:, 2 * p : 2 * p + 2, 1, :],
                                hg8[:, 2 * p : 2 * p + 2, 0, lts],
                                start=False,
                                stop=(p == HB // 2 - 1),
                                perf_mode=DR,
                            )
                    for ti in range(TI):
                        # the very last slab (tg1, e3, cb5, ti1) drains in
                        # 256-col pieces so the post-PE critical chain
                        # (DVE mul+add -> out DMA -> drain) is short
                        last = tg == TCG - 1 and e == EL - 1 and cb == CB - 1 and ti == TI - 1
                        pieces = 2 if last else 1
                        for pi in range(pieces):
                            w = 512 // pieces
                            gts = slice(
                                tg * TG + ti * 512 + pi * w,
                                tg * TG + ti * 512 + (pi + 1) * w,
                            )
                            lts = slice(ti * 512 + pi * w, ti * 512 + (pi + 1) * w)
                            pts = slice(pi * w, (pi + 1) * w)
                            if e == 0:
                                nc.vector.tensor_mul(
                                    yac[:, cb, lts], g_sb[:, 0, gts], yps[ti][:, pts]
                                )
                            else:
                                yt = sp.tile([128, 512], F32, tag="yt", bufs=3, name="yt")
                                nc.vector.tensor_mul(
                                    yt[:, pts], g_sb[:, e, gts], yps[ti][:, pts]
                                )
                                nc.vector.tensor_add(
                                    yac[:, cb, lts], yt[:, pts], yac[:, cb, lts]
                                )
                            if e == EL - 1:
                                nc.sync.dma_start(
                                    outT_d[cb * 128 : (cb + 1) * 128, gts],
                                    yac[:, cb, lts],
                                )

            # --- main ---
            for tg in range(TCG):
                hg8 = sp.tile([128, HB, 2, TG], F8, tag="hg", bufs=1, name="hg8")
                yac = sp.tile([128, CB, TG], F32, tag="yacc", bufs=1, name="yac")
                for e in range(EL):
                    if tg == 0 and e == 0:
                        emit_mm1_first(hg8)
                        emit_mm1(tg, e, hg8, hbg_start=1)
                        # deferred bulk transfers: queued behind e0's mm1
                        # weight tiles, ahead of everything they feed
                        for j in range(EL):
                            nc.sync.dma_start(
                                g_sb[:, j, :], g_d[j : j + 1, :].to_broadcast((128, N))
                            )
                        nc.sync.dma_start(x8_sb[:, 1, :, :, :], x8_d[:, 1, :, :, :])
                    else:
                        emit_mm1(tg, e, hg8)
                    emit_mm2(tg, e, hg8, yac)

    nc.compile()
    return nc


def _get_nc():
    global _CACHED_NC
    if _CACHED_NC is None:
        _CACHED_NC = build_nc()
    return _CACHED_NC


def _q8(a):
    return np.clip(a, -240.0, 240.0).astype(E4NP)


def make_in_maps(x, gate_w, gate_b, w1, b1, w2, b2):
    x = np.asarray(x, np.float32).reshape(N, C)
    gate_w = np.asarray(gate_w, np.float32)
    gate_b = np.asarray(gate_b, np.float32)
    w1 = np.asarray(w1, np.float32)
    b1 = np.asarray(b1, np.float32)
    w2 = np.asarray(w2, np.float32)

    # host-side gates (fp64 softmax)
    z = x.astype(np.float64) @ gate_w.astype(np.float64) + gate_b.astype(np.float64)
    ge = np.exp(z - z.max(-1, keepdims=True))
    gates = (ge / ge.sum(-1, keepdims=True)).astype(np.float32)  # [N, E]

    # x split -> packed [128, tg, cc, 2(hi,lo), TG]
    xT = np.ascontiguousarray(x.T)  # [C, N]
    xh = _q8(xT)
    xl = _q8(xT - xh.astype(np.float32))
    x8 = np.stack([xh, xl], axis=1)  # [C, 2, N]
    x8 = x8.reshape(CB, 128, 2, TCG, TG).transpose(1, 3, 0, 2, 4)
    x8 = np.ascontiguousarray(x8)  # [128, tg, cc, 2, TG]

    # w1 split (scaled by S1) -> packed [E, hbg, 128, cc*2(lo,hi)*512]
    w1s = S1 * w1
    w1h = _q8(w1s)
    w1l = _q8(w1s - w1h.astype(np.float32))
    w1_8 = np.stack([w1l, w1h], axis=2)  # [E, C, 2, H]
    w1_8 = (
        w1_8.reshape(E, CB, 128, 2, HB // 4, 512)
        .transpose(0, 4, 2, 1, 3, 5)
        .reshape(E, HB // 4, 128, CB * 2 * 512)
    )
    w1_8 = np.ascontiguousarray(w1_8)

    # w2 split (scaled by S2) -> packed [E, cb, 128, hb*2(lo,hi)*128]
    w2s = S2 * w2
    w2h = _q8(w2s)
    w2l = _q8(w2s - w2h.astype(np.float32))
    w2_8 = np.stack([w2l, w2h], axis=2)  # [E, H, 2, C]
    w2_8 = (
        w2_8.reshape(E, HB, 128, 2, CB, 128)
        .transpose(0, 4, 2, 1, 3, 5)
        .reshape(E, CB, 128, HB * 2 * 128)
    )
    w2_8 = np.ascontiguousarray(w2_8)

    g_scaled = (gates / S2).astype(np.float16)  # [N, E]

    in_maps = []
    for i in range(NCORES):
        lo, hi = EL * i, EL * (i + 1)
        in_maps.append(
            {
                "x8": x8,
                "g": np.ascontiguousarray(g_scaled[:, lo:hi].T),
                "w1": w1_8[lo:hi],
                "b1": np.ascontiguousarray(
                    b1[lo:hi].reshape(EL, HB, 128).transpose(2, 0, 1)
                ),
                "w2": w2_8[lo:hi],
            }
        )
    return in_maps, gates


def kernel(x, gate_w, gate_b, w1, b1, w2, b2, _trace=False, _tmpdir=None):
    nc = _get_nc()
    in_maps, gates = make_in_maps(x, gate_w, gate_b, w1, b1, w2, b2)
    res = run_bass_kernel_spmd(
        nc,
        in_maps,
        core_ids=list(range(NCORES)),
        trace=_trace,
        tmpdir=_tmpdir,
    )
    acc = res.results[0]["outT"].astype(np.float64)
    for r in res.results[1:]:
        acc += r["outT"]
    out = acc.T
    # gate-weighted b2 term, host-side (b2 is zero for this problem's inputs)
    b2 = np.asarray(b2, np.float64)
    if np.any(b2):
        out = out + gates.astype(np.float64) @ b2
    out = out.reshape(B, T, C).astype(np.float32)
    if _trace:
        kernel._last_results = res
    return out


# revision 17
# speedup vs baseline: 1.0022x; 1.0022x over previous
"""MoE CouncilLayer kernel for 8x TRN2 NeuronCores (expert-parallel).

Problem (all-expert MoE, B=2, T=1024, C=768, E=32, H=3072):
    gates = softmax(x @ gate_w + gate_b)                     # [N, E]
    h     = gelu(einsum('nc,ech->neh', x, w1) + b1)          # [N, E, H]
    y     = einsum('neh,ehc->nec', h, w2) + b2               # [N, E, C]
    out   = einsum('ne,nec->nc', gates, y)                   # [N, C]

Sharding: expert-parallel, 4 experts per core; x replicated. Each core
computes its 4 experts' gate-weighted partial sum; host adds the 8
partials. Gates are computed host-side (they are needed on host anyway
to weight b2, and they are off the device critical path entirely).

Arithmetic: 3-term hi/lo fp8 with DoubleRow perf mode. Every matmul
operand A is split host-side (or on-device for h) into e4m3 hi+lo
parts, A = Ah + Al + O(0.07% of A). The product A@B is computed as
    Ah@Bh  +  (Al@Bh + Ah@Bl)        [lo@lo dropped, ~1e-4 relative]
where the two cross terms fit in ONE DoubleRow matmul instruction
(DoubleRow computes lhsT[:,0].T @ rhs[:,0] + lhsT[:,1].T @ rhs[:,1]),
and hi@hi terms pair ADJACENT contraction chunks in one instruction.
Net: 1.5 DoubleRow instructions per 128-contraction chunk, at 0.5
cycles/output-column each = 0.75x the fp16 PE cost. End-to-end rel
err ~2e-3 (vs 2e-2 budget): mm1/mm2 operand residuals ~0.07% each.

Scaling: w1 is scaled by 32 and w2 by 64 host-side so the lo residues
land in e4m3 normal range; mm1 descales via the ACT gelu scale
(gelu(psum/32 + b1)) and mm2 descales via host-prescaled gates
(g/64), applied after mm2 on DVE:  yac += (g/64) * yps.

Per-core layout is feature-major (activations stored [feature, token]).
hi/lo parts are interleaved in one tile dim so both DoubleRow operand
pairings are single strided APs:
    x8  [128, cc, 2, N]   dim2: 0=hi 1=lo
    w1t [128, cc, 2, 512] dim2: 0=lo 1=hi   (cross pair = [:,cc,0:2,:])
    hg8 [128, hb, 2, TG]  dim2: 0=hi 1=lo
    w2t [128, hb, 2, 128] dim2: 0=lo 1=hi
h split: ACT emits gelu twice from the mm1 psum (fp16 tmp + fp8 hi),
DVE tensor_sub produces the fp8 lo. Engine budget per core: PE ~738us
(bottleneck), ACT ~490us, DVE ~370us, DMA ~44MB ~130us.
"""

import numpy as np
import ml_dtypes

import concourse.tile as tile
from concourse import bacc, mybir
from concourse.bass_utils import run_bass_kernel_spmd

# Problem dims (hardcoded per harness contract)
B, T, C, E, H = 2, 1024, 768, 32, 3072
N = B * T  # 2048 tokens
NCORES = 8
EL = E // NCORES  # 4 local experts
CB = C // 128  # 6 c-blocks (3 DoubleRow pairs)
HB = H // 128  # 24 h-blocks (12 DoubleRow pairs)
TCG = 2  # token groups (1024 each)
TG = N // TCG  # 1024
TI = TG // 512  # 512-token chunks per group

S1 = 32.0  # host-side w1 scale
S2 = 64.0  # host-side w2 scale

F8 = mybir.dt.float8e4
F16 = mybir.dt.float16
F32 = mybir.dt.float32
AF = mybir.ActivationFunctionType
DR = mybir.MatmulPerfMode.DoubleRow
E4NP = ml_dtypes.float8_e4m3

_CACHED_NC = None


def build_nc(act=AF.Gelu):
    nc = bacc.Bacc(trn_type="TRN2")

    # tile-layout packed inputs (host pre-arranges so DMAs are contiguous):
    # x8: [128, tg, cc, 2(hi,lo), TG];  w1: [EL, hbg, 128, cc*2(lo,hi)*512]
    # w2: [EL, cb, 128, hb*2(lo,hi)*128]
    x8_d = nc.dram_tensor("x8", [128, TCG, CB, 2, TG], F8, kind="ExternalInput")
    g_d = nc.dram_tensor("g", [EL, N], F16, kind="ExternalInput")
    w1_d = nc.dram_tensor("w1", [EL, HB // 4, 128, CB * 2 * 512], F8, kind="ExternalInput")
    b1_d = nc.dram_tensor("b1", [128, EL, HB], F32, kind="ExternalInput")
    w2_d = nc.dram_tensor("w2", [EL, CB, 128, HB * 2 * 128], F8, kind="ExternalInput")
    outT_d = nc.dram_tensor("outT", [C, N], F32, kind="ExternalOutput")

    with tile.TileContext(nc) as tc:
        with (
            tc.tile_pool(name="const", bufs=1) as cp,
            tc.tile_pool(name="stream", bufs=1) as sp,
            tc.tile_pool(name="psum", bufs=1, space="PSUM") as pp,
        ):
            # --- resident tiles ---
            x8_sb = cp.tile([128, TCG, CB, 2, TG], F8)
            g_sb = cp.tile([128, EL, N], F16)
            b1_sb = cp.tile([128, EL, HB], F32)

            # DMA issue order = arrival order. The specially-traced first
            # mm1 block consumes x cc-chunks as they land; its w1 tile and
            # the first x chunks go first, b1 (needed by the first gelu)
            # rides after the first group, then the rest of x tg0, the
            # second w1 tile prefetch, x tg1, and the gate broadcasts.
            w1t_first = sp.tile([128, CB, 2, 512], F8, tag="w1", bufs=4, name="w1t")
            w1f_ap = w1_d[0, 0, :, :].rearrange(
                "p (cc two h) -> p cc two h", cc=CB, two=2
            )
            # finest-first interleave: the opening matmul needs only x cc0
            # (first 512 tokens) + w1 cc0; each later group keeps PE just
            # behind the arrival stream through the cc-outer first block.
            nc.sync.dma_start(x8_sb[:, 0, 0, :, 0:512], x8_d[:, 0, 0, :, 0:512])
            nc.sync.dma_start(w1t_first[:, 0, :, :], w1f_ap[:, 0, :, :])
            nc.sync.dma_start(x8_sb[:, 0, 0, :, 512:TG], x8_d[:, 0, 0, :, 512:TG])
            nc.sync.dma_start(x8_sb[:, 0, 1, :, :], x8_d[:, 0, 1, :, :])
            nc.sync.dma_start(w1t_first[:, 1, :, :], w1f_ap[:, 1, :, :])
            nc.sync.dma_start(x8_sb[:, 0, 2:4, :, :], x8_d[:, 0, 2:4, :, :])
            nc.sync.dma_start(w1t_first[:, 2:4, :, :], w1f_ap[:, 2:4, :, :])
            nc.sync.dma_start(x8_sb[:, 0, 4:6, :, :], x8_d[:, 0, 4:6, :, :])
            nc.sync.dma_start(w1t_first[:, 4:6, :, :], w1f_ap[:, 4:6, :, :])
            nc.sync.dma_start(b1_sb, b1_d[:, :, :])
            # prefetch e0's second w1 tile; x tg1 and the gate broadcasts are
            # deferred into the main loop (they are not needed until ~50us in
            # and would delay e0's hbg2+ weight tiles here).
            w1t_second = sp.tile([128, CB, 2, 512], F8, tag="w1", bufs=4, name="w1t")
            nc.sync.dma_start(w1t_second, w1_d[0, 1, :, :])

            def emit_mm1_tile(ps, w1t, hbi, tg, lts, start_first):
                # 9 DoubleRow instrs: 6 cross (per cc) + 3 hi-hi (per pair)
                hs = slice(hbi * 128, (hbi + 1) * 128)
                for cc in range(CB):
                    nc.tensor.matmul(
                        ps,
                        w1t[:, cc, 0:2, hs],
                        x8_sb[:, tg, cc, 0:2, lts],
                        start=(cc == 0) and start_first,
                        stop=False,
                        perf_mode=DR,
                    )
                for p in range(CB // 2):
                    nc.tensor.matmul(
                        ps,
                        w1t[:, 2 * p : 2 * p + 2, 1, hs],
                        x8_sb[:, tg, 2 * p : 2 * p + 2, 0, lts],
                        start=False,
                        stop=(p == CB // 2 - 1),
                        perf_mode=DR,
                    )

            def emit_gelu_split(tg, e, hb, hps, hg8):
                # ACT: gelu from psum -> fp16; Pool(gpsimd): cast fp16 -> fp8
                # hi part; DVE: subtract -> fp8 lo part. One pass per engine
                # per tile keeps all three under PE's 960ns/tile mm1 pace.
                for ti in range(TI):
                    lts = slice(ti * 512, (ti + 1) * 512)
                    h16 = sp.tile([128, 512], F16, tag="h16", bufs=8, name="h16")
                    nc.scalar.activation(
                        h16, hps[ti], act, bias=b1_sb[:, e, hb : hb + 1], scale=1.0 / S1
                    )
                    nc.gpsimd.tensor_copy(hg8[:, hb, 0, lts], h16)
                    nc.vector.tensor_sub(hg8[:, hb, 1, lts], h16, hg8[:, hb, 0, lts])

            def emit_mm1_first(hg8):
                # first 4 h-blocks of (tg0, e0), traced cc-OUTER across all 8
                # psum banks so PE consumes each arriving x chunk immediately;
                # hi-hi terms stitched in after each odd cc completes a pair.
                hps8 = [
                    [
                        pp.tile(
                            [128, 512],
                            F32,
                            tag=("h" if hbi < 2 else "y"),
                            bufs=4,
                            name="hps",
                        )
                        for _ in range(TI)
                    ]
                    for hbi in range(4)
                ]
                for cc in range(CB):
                    for ti in range(TI):
                        lts = slice(ti * 512, (ti + 1) * 512)
                        for hbi in range(4):
                            nc.tensor.matmul(
                                hps8[hbi][ti],
                                w1t_first[:, cc, 0:2, hbi * 128 : (hbi + 1) * 128],
                                x8_sb[:, 0, cc, 0:2, lts],
                                start=(cc == 0),
                                stop=False,
                                perf_mode=DR,
                            )
                    if cc % 2 == 1:
                        p = cc // 2
                        for ti in range(TI):
                            lts = slice(ti * 512, (ti + 1) * 512)
                            for hbi in range(4):
                                nc.tensor.matmul(
                                    hps8[hbi][ti],
                                    w1t_first[:, cc - 1 : cc + 1, 1, hbi * 128 : (hbi + 1) * 128],
                                    x8_sb[:, 0, cc - 1 : cc + 1, 0, lts],
                                    start=False,
                                    stop=(p == CB // 2 - 1),
                                    perf_mode=DR,
                                )
                for hbi in range(4):
                    emit_gelu_split(0, 0, hbi, hps8[hbi], hg8)

            def emit_mm1(tg, e, hg8, hbg_start=0):
                for hbg in range(hbg_start, HB // 4):
                    if tg == 0 and e == 0 and hbg == 1:
                        w1t = w1t_second
                    else:
                        w1t = sp.tile(
                            [128, CB, 2, 512], F8, tag="w1", bufs=4, name="w1t"
                        )
                        nc.sync.dma_start(w1t, w1_d[e, hbg, :, :])
                    for hbi in range(4):
                        hb = hbg * 4 + hbi
                        hps = [
                            pp.tile([128, 512], F32, tag="h", bufs=4, name="hps")
                            for _ in range(TI)
                        ]
                        for ti in range(TI):
                            lts = slice(ti * 512, (ti + 1) * 512)
                            emit_mm1_tile(hps[ti], w1t, hbi, tg, lts, True)
                        emit_gelu_split(tg, e, hb, hps, hg8)

            def emit_mm2(tg, e, hg8, yac):
                for cb in range(CB):
                    w2t = sp.tile([128, HB, 2, 128], F8, tag="w2", bufs=3, name="w2t")
                    nc.sync.dma_start(w2t, w2_d[e, cb, :, :])
                    # the very last slab (tg1, e3, cb5) splits its second
                    # half into 256-col psum chunks so the post-PE critical
                    # chain (DVE mul+add -> out DMA -> drain) covers only
                    # 256 columns; the earlier chunks' DVE/DMA overlap the
                    # final chunk's matmuls.
                    last = tg == TCG - 1 and e == EL - 1 and cb == CB - 1
                    chunks = [(0, 512), (512, 256), (768, 256)] if last else [
                        (0, 512), (512, 512)
                    ]
                    for off, w in chunks:
                        ypc = pp.tile([128, w], F32, tag="y", bufs=4, name="yps")
                        lts = slice(off, off + w)
                        for hb in range(HB):
                            nc.tensor.matmul(
                                ypc,
                                w2t[:, hb, 0:2, :],
                                hg8[:, hb, 0:2, lts],
                                start=(hb == 0),
                                stop=False,
                                perf_mode=DR,
                            )
                        for p in range(HB // 2):
                            nc.tensor.matmul(
                                ypc,
                                w2t[:, 2 * p : 2 * p + 2, 1, :],
                                hg8[:, 2 * p : 2 * p + 2, 0, lts],
                                start=False,
                                stop=(p == HB // 2 - 1),
                                perf_mode=DR,
                            )
                        gts = slice(tg * TG + off, tg * TG + off + w)
                        if e == 0:
                            nc.vector.tensor_mul(yac[:, cb, lts], g_sb[:, 0, gts], ypc)
                        else:
                            yt = sp.tile([128, 512], F32, tag="yt", bufs=3, name="yt")
                            nc.vector.tensor_mul(yt[:, 0:w], g_sb[:, e, gts], ypc)
                            nc.vector.tensor_add(
                                yac[:, cb, lts], yt[:, 0:w], yac[:, cb, lts]
                            )
                        if e == EL - 1:
                            nc.sync.dma_start(
                                outT_d[cb * 128 : (cb + 1) * 128, gts],
                                yac[:, cb, lts],
                            )

            # --- main ---
            for tg in range(TCG):
                hg8 = sp.tile([128, HB, 2, TG], F8, tag="hg", bufs=1, name="hg8")
                yac = sp.tile([128, CB, TG], F32, tag="yacc", bufs=1, name="yac")
                for e in range(EL):
                    if tg == 0 and e == 0:
                        emit_mm1_first(hg8)
                        emit_mm1(tg, e, hg8, hbg_start=1)
                        # deferred bulk transfers: queued behind e0's mm1
                        # weight tiles, ahead of everything they feed
                        for j in range(EL):
                            nc.sync.dma_start(
                                g_sb[:, j, :], g_d[j : j + 1, :].to_broadcast((128, N))
                            )
                        nc.sync.dma_start(x8_sb[:, 1, :, :, :], x8_d[:, 1, :, :, :])
                    else:
                        emit_mm1(tg, e, hg8)
                    emit_mm2(tg, e, hg8, yac)

    nc.compile()
    return nc


def _get_nc():
    global _CACHED_NC
    if _CACHED_NC is None:
        _CACHED_NC = build_nc()
    return _CACHED_NC


def _q8(a):
    return np.clip(a, -240.0, 240.0).astype(E4NP)


def make_in_maps(x, gate_w, gate_b, w1, b1, w2, b2):
    x = np.asarray(x, np.float32).reshape(N, C)
    gate_w = np.asarray(gate_w, np.float32)
    gate_b = np.asarray(gate_b, np.float32)
    w1 = np.asarray(w1, np.float32)
    b1 = np.asarray(b1, np.float32)
    w2 = np.asarray(w2, np.float32)

    # host-side gates (fp64 softmax)
    z = x.astype(np.float64) @ gate_w.astype(np.float64) + gate_b.astype(np.float64)
    ge = np.exp(z - z.max(-1, keepdims=True))
    gates = (ge / ge.sum(-1, keepdims=True)).astype(np.float32)  # [N, E]

    # x split -> packed [128, tg, cc, 2(hi,lo), TG]
    xT = np.ascontiguousarray(x.T)  # [C, N]
    xh = _q8(xT)
    xl = _q8(xT - xh.astype(np.float32))
    x8 = np.stack([xh, xl], axis=1)  # [C, 2, N]
    x8 = x8.reshape(CB, 128, 2, TCG, TG).transpose(1, 3, 0, 2, 4)
    x8 = np.ascontiguousarray(x8)  # [128, tg, cc, 2, TG]

    # w1 split (scaled by S1) -> packed [E, hbg, 128, cc*2(lo,hi)*512]
    w1s = S1 * w1
    w1h = _q8(w1s)
    w1l = _q8(w1s - w1h.astype(np.float32))
    w1_8 = np.stack([w1l, w1h], axis=2)  # [E, C, 2, H]
    w1_8 = (
        w1_8.reshape(E, CB, 128, 2, HB // 4, 512)
        .transpose(0, 4, 2, 1, 3, 5)
        .reshape(E, HB // 4, 128, CB * 2 * 512)
    )
    w1_8 = np.ascontiguousarray(w1_8)

    # w2 split (scaled by S2) -> packed [E, cb, 128, hb*2(lo,hi)*128]
    w2s = S2 * w2
    w2h = _q8(w2s)
    w2l = _q8(w2s - w2h.astype(np.float32))
    w2_8 = np.stack([w2l, w2h], axis=2)  # [E, H, 2, C]
    w2_8 = (
        w2_8.reshape(E, HB, 128, 2, CB, 128)
        .transpose(0, 4, 2, 1, 3, 5)
        .reshape(E, CB, 128, HB * 2 * 128)
    )
    w2_8 = np.ascontiguousarray(w2_8)

    g_scaled = (gates / S2).astype(np.float16)  # [N, E]

    in_maps = []
    for i in range(NCORES):
        lo, hi = EL * i, EL * (i + 1)
        in_maps.append(
            {
                "x8": x8,
                "g": np.ascontiguousarray(g_scaled[:, lo:hi].T),
                "w1": w1_8[lo:hi],
                "b1": np.ascontiguousarray(
                    b1[lo:hi].reshape(EL, HB, 128).transpose(2, 0, 1)
                ),
                "w2": w2_8[lo:hi],
            }
        )
    return in_maps, gates


def kernel(x, gate_w, gate_b, w1, b1, w2, b2, _trace=False, _tmpdir=None):
    nc = _get_nc()
    in_maps, gates = make_in_maps(x, gate_w, gate_b, w1, b1, w2, b2)
    res = run_bass_kernel_spmd(
        nc,
        in_maps,
        core_ids=list(range(NCORES)),
        trace=_trace,
        tmpdir=_tmpdir,
    )
    acc = res.results[0]["outT"].astype(np.float64)
    for r in res.results[1:]:
        acc += r["outT"]
    out = acc.T
    # gate-weighted b2 term, host-side (b2 is zero for this problem's inputs)
    b2 = np.asarray(b2, np.float64)
    if np.any(b2):
        out = out + gates.astype(np.float64) @ b2
    out = out.reshape(B, T, C).astype(np.float32)
    if _trace:
        kernel._last_results = res
    return out
